# revision 2
# baseline (speedup 1.0000x reference)
"""Tensor-parallel LlamaAttention (S=2048, HID=4096, NH=32, NKV=8) on 8 trn2 cores.

Sharding: core c owns q heads {c, c+8, c+16, c+24} (head h uses kv head h%8,
so all four share kv head c) and kv head c.  Projections + attention are fully
local; avT (bf16, [128d, 2048s] per head) is AllGathered per head-group, then
each core computes its 512 output columns of o_proj (column-parallel wo).

v2 layout: interleaved startup DMAs (x chunk0 with wq), phase-2 batched into
scores->exp->rowsum->av sub-batches with width-restricted diagonal tiles (no
memsets), deferred normalization, per-chunk AllGather pieces for the last head
group, and arrival-ordered phase-3 accumulation (last group's matmuls last in
each chain) with quartered agt tiles so o_proj overlaps the collectives.

Self-contained: shapes/sharding hardcoded; host does transposes/casts.
"""

import numpy as np
import ml_dtypes

import concourse.bacc as bacc
import concourse.tile as tile
import concourse.mybir as mybir
from concourse.bass_utils import run_bass_kernel_spmd

S = 2048
HID = 4096
NH = 32
NKV = 8
HD = 128
HALF = 64
N_CORES = 8
NREP = NH // NKV  # 4 q heads per core
NHT = HID // 128  # 32 hidden tiles
NST = S // 128    # 16 seq tiles
NSC = S // 512    # 4 seq chunks
BF16 = mybir.dt.bfloat16
F32 = mybir.dt.float32

_CACHE = {}


def build_nc():
    nc = bacc.Bacc("TRN2", target_bir_lowering=False, debug=False,
                   num_devices=N_CORES)

    xT = nc.dram_tensor("xT", [HID, S], BF16, kind="ExternalInput").ap()
    wq = nc.dram_tensor("wqT", [HID, NREP * HD], BF16, kind="ExternalInput").ap()
    wk = nc.dram_tensor("wkT", [HID, HD], BF16, kind="ExternalInput").ap()
    wv = nc.dram_tensor("wvT", [HID, HD], BF16, kind="ExternalInput").ap()
    wo = nc.dram_tensor("woT", [HID, 512], BF16, kind="ExternalInput").ap()
    cosT = nc.dram_tensor("cosT", [HD, S], F32, kind="ExternalInput").ap()
    sinT = nc.dram_tensor("sinT", [HD, S], F32, kind="ExternalInput").ap()
    tri = nc.dram_tensor("triT", [128, 128], BF16, kind="ExternalInput").ap()
    ones_c = nc.dram_tensor("ones_c", [128, 1], BF16, kind="ExternalInput").ap()
    ones_r = nc.dram_tensor("ones_r", [1, 128], F32, kind="ExternalInput").ap()

    o_out = nc.dram_tensor("o_out", [S, 512], F32, kind="ExternalOutput").ap()

    # head groups 0..2: one AllGather each over the full [128, S] avT
    ag_in = [nc.dram_tensor(f"ag_in{j}", [HD, S], BF16).ap() for j in range(3)]
    ag_out = [nc.dram_tensor(f"ag_out{j}", [N_CORES * HD, S], BF16,
                             addr_space="Shared").ap() for j in range(3)]
    # head group 3: per-chunk pieces so the tail exposure is one small piece
    ag3_in = [nc.dram_tensor(f"ag3_in{c}", [HD, 512], BF16).ap()
              for c in range(NSC)]
    ag3_out = [nc.dram_tensor(f"ag3_out{c}", [N_CORES * HD, 512], BF16,
                              addr_space="Shared").ap() for c in range(NSC)]

    with tile.TileContext(nc) as tc:
        _body(nc, tc, xT, wq, wk, wv, wo, cosT, sinT, tri, ones_c, ones_r,
              o_out, ag_in, ag_out, ag3_in, ag3_out)
    nc.compile()
    return nc


def _body(nc, tc, xT, wq, wk, wv, wo, cosT, sinT, tri, ones_c, ones_r,
          o_out, ag_in, ag_out, ag3_in, ag3_out):
    with (
        tc.tile_pool(name="consts", bufs=1) as cpool,
        tc.tile_pool(name="psum", bufs=4, space="PSUM") as psum,
        tc.tile_pool(name="psav", bufs=2, space="PSUM") as psav,
        tc.tile_pool(name="psrs", bufs=2, space="PSUM") as psrs,
    ):
        tri_sb = cpool.tile([128, 128], BF16, tag="tri")
        onc_sb = cpool.tile([128, 1], BF16, tag="onc")
        onr_sb = cpool.tile([1, 128], F32, tag="onr")
        nc.sync.dma_start(out=tri_sb[:], in_=tri[:])
        nc.sync.dma_start(out=onc_sb[:], in_=ones_c[:])
        nc.sync.dma_start(out=onr_sb[:], in_=ones_r[:])

        with tc.tile_pool(name="qkv", bufs=1) as qkvpool:
            qT_sb = [qkvpool.tile([HD, S], BF16, tag=f"qT{j}", name=f"qT{j}")
                     for j in range(NREP)]
            kT_sb = qkvpool.tile([HD, S], BF16, tag="kT")
            v_sb = qkvpool.tile([128, S], BF16, tag="v")  # col block kt = s tile kt

            with (
                tc.tile_pool(name="rconsts", bufs=1) as rcpool,
                tc.tile_pool(name="wproj", bufs=1) as wpool,
                tc.tile_pool(name="xc", bufs=64) as xpool,
                tc.tile_pool(name="rope", bufs=2) as rpool,
            ):
                _phase1(nc, tc, xT, wq, wk, wv, cosT, sinT, qT_sb, kT_sb, v_sb,
                        rcpool, wpool, xpool, rpool, psum)

            with (
                tc.tile_pool(name="probs", bufs=18) as ppool,
                tc.tile_pool(name="avc", bufs=2) as avcpool,
                tc.tile_pool(name="small", bufs=2) as spool,
                tc.tile_pool(name="wo", bufs=1) as wopool,
                tc.tile_pool(name="ag", bufs=3) as agpool,
                tc.tile_pool(name="oout", bufs=2) as opool,
            ):
                # o_proj weights prefetch during phase 2
                wo_sb = wopool.tile([128, NHT * 512], BF16, tag="wo")
                for i in range(NHT):
                    nc.sync.dma_start(out=wo_sb[:, i * 512:(i + 1) * 512],
                                      in_=wo[i * 128:(i + 1) * 128, :])

                agq = {}
                _phase2(nc, tc, qT_sb, kT_sb, v_sb, tri_sb, onc_sb, onr_sb,
                        ag_in, ag_out, ag3_in, ag3_out, agq, agpool,
                        ppool, avcpool, spool, psum, psav, psrs)
                _phase3(nc, tc, wo_sb, o_out, ag_out, agq, agpool, psum, opool)


def _phase1(nc, tc, xT, wq, wk, wv, cosT, sinT, qT_sb, kT_sb, v_sb,
            rcpool, wpool, xpool, rpool, psum):
    cos_sb = rcpool.tile([HD, S], F32, tag="cos")
    sin_sb = rcpool.tile([HD, S], F32, tag="sin")

    wq_sb = wpool.tile([128, NHT * 512], BF16, tag="wq")
    wk_sb = wpool.tile([128, NHT * 128], BF16, tag="wk")
    wv_sb = wpool.tile([128, NHT * 128], BF16, tag="wv")

    # -- DMA issue order tuned for startup: wq and x chunk0 interleaved so the
    #    first q accumulation chains start within a few us, then wk/wv, then
    #    cos/sin (first needed ~30us in, after the first q chains), then the
    #    rest of x (2-chunk prefetch window via pool rotation).
    xcs = {}
    for h in range(NHT):
        nc.sync.dma_start(out=wq_sb[:, h * 512:(h + 1) * 512],
                          in_=wq[h * 128:(h + 1) * 128, :])
        t = xpool.tile([128, 512], BF16, tag="xc", name=f"xc0_{h}")
        nc.sync.dma_start(out=t[:], in_=xT[h * 128:(h + 1) * 128, 0:512])
        xcs[(0, h)] = t
    for h in range(NHT):
        nc.sync.dma_start(out=wk_sb[:, h * 128:(h + 1) * 128],
                          in_=wk[h * 128:(h + 1) * 128, :])
        nc.sync.dma_start(out=wv_sb[:, h * 128:(h + 1) * 128],
                          in_=wv[h * 128:(h + 1) * 128, :])
    for cs in range(NSC):
        sc = slice(cs * 512, (cs + 1) * 512)
        nc.sync.dma_start(out=cos_sb[:, sc], in_=cosT[:, sc])
        nc.sync.dma_start(out=sin_sb[:, sc], in_=sinT[:, sc])
    for cs in range(1, NSC):
        for h in range(NHT):
            t = xpool.tile([128, 512], BF16, tag="xc", name=f"xc{cs}_{h}")
            nc.sync.dma_start(out=t[:],
                              in_=xT[h * 128:(h + 1) * 128,
                                     cs * 512:(cs + 1) * 512])
            xcs[(cs, h)] = t

    for cs in range(NSC):
        sc = slice(cs * 512, (cs + 1) * 512)

        def _rope(dst, pp):
            t1 = rpool.tile([HALF, 512], F32, tag="t1")
            t2 = rpool.tile([HALF, 512], F32, tag="t2")
            nc.vector.tensor_mul(t1[:], pp[0:HALF, :], cos_sb[0:HALF, sc])
            nc.vector.tensor_mul(t2[:], pp[HALF:128, :], sin_sb[0:HALF, sc])
            nc.vector.tensor_sub(dst[0:HALF, sc], t1[:], t2[:])
            t3 = rpool.tile([HALF, 512], F32, tag="t1")
            t4 = rpool.tile([HALF, 512], F32, tag="t2")
            nc.vector.tensor_mul(t3[:], pp[HALF:128, :], cos_sb[HALF:128, sc])
            nc.vector.tensor_mul(t4[:], pp[0:HALF, :], sin_sb[HALF:128, sc])
            nc.vector.tensor_add(dst[HALF:128, sc], t3[:], t4[:])

        for j in range(NREP):
            pq = psum.tile([128, 512], F32, tag="mm")
            for h in range(NHT):
                nc.tensor.matmul(
                    pq[:],
                    wq_sb[:, h * 512 + j * 128: h * 512 + (j + 1) * 128],
                    xcs[(cs, h)][:],
                    start=(h == 0), stop=(h == NHT - 1))
            _rope(qT_sb[j], pq)

        pk = psum.tile([128, 512], F32, tag="mm")
        for h in range(NHT):
            nc.tensor.matmul(pk[:], wk_sb[:, h * 128:(h + 1) * 128],
                             xcs[(cs, h)][:],
                             start=(h == 0), stop=(h == NHT - 1))
        _rope(kT_sb, pk)

        pv = psum.tile([128, 512], F32, tag="mm")
        for tl in range(4):
            for h in range(NHT):
                nc.tensor.matmul(
                    pv[:, tl * 128:(tl + 1) * 128],
                    xcs[(cs, h)][:, tl * 128:(tl + 1) * 128],
                    wv_sb[:, h * 128:(h + 1) * 128],
                    start=(h == 0), stop=(h == NHT - 1))
        nc.scalar.copy(v_sb[:, sc], pv[:])


def _phase2(nc, tc, qT_sb, kT_sb, v_sb, tri_sb, onc_sb, onr_sb,
            ag_in, ag_out, ag3_in, ag3_out, agq, agpool,
            ppool, avcpool, spool, psum, psav, psrs):
    Exp = mybir.ActivationFunctionType.Exp
    carry = [None]

    def make_norm(j, C, pav, prs):
        def f():
            qc = slice(C * 512, (C + 1) * 512)
            rrec = spool.tile([1, 512], F32, tag="rrec", name=f"rrec{j}_{C}")
            nc.vector.reciprocal(rrec[:], prs[:])
            pb = psum.tile([128, 512], F32, tag="mm", name=f"pb{j}_{C}")
            nc.tensor.matmul(pb[:], onr_sb[:], rrec[:], start=True, stop=True)
            bsb = spool.tile([128, 512], F32, tag="bsb", name=f"bsb{j}_{C}")
            nc.scalar.copy(bsb[:], pb[:])
            avc = avcpool.tile([128, 512], BF16, tag="avc", name=f"avc{j}_{C}")
            nc.vector.tensor_mul(avc[:], pav[:], bsb[:])
            if j < 3:
                nc.sync.dma_start(out=ag_in[j][:, qc], in_=avc[:])
                if C == NSC - 1:
                    nc.gpsimd.collective_compute(
                        "AllGather", mybir.AluOpType.bypass,
                        replica_groups=[list(range(N_CORES))],
                        ins=[ag_in[j][:]], outs=[ag_out[j][:]])
                    # prefetch quarters 0-1 of this group's agt tiles
                    for qq in range(2):
                        for r in range(N_CORES):
                            t = agpool.tile([128, 512], BF16, tag=f"ag{j}_{r}",
                                            name=f"ag{j}_{r}_{qq}")
                            nc.sync.dma_start(
                                out=t[:],
                                in_=ag_out[j][r * 128:(r + 1) * 128,
                                              qq * 512:(qq + 1) * 512])
                            agq[(j, r, qq)] = t
            else:
                nc.sync.dma_start(out=ag3_in[C][:], in_=avc[:])
                nc.gpsimd.collective_compute(
                    "AllGather", mybir.AluOpType.bypass,
                    replica_groups=[list(range(N_CORES))],
                    ins=[ag3_in[C][:]], outs=[ag3_out[C][:]])
                for r in range(N_CORES):
                    t = agpool.tile([128, 512], BF16, tag=f"ag3_{r}",
                                    name=f"ag3_{r}_{C}")
                    nc.sync.dma_start(
                        out=t[:], in_=ag3_out[C][r * 128:(r + 1) * 128, :])
                    agq[(3, r, C)] = t
        return f

    for j in range(NREP):
        for C in range(NSC):
            qc0 = C * 512
            nkt = 4 * C + 4
            pts = []
            # -- scores + exp batch (PE: contiguous scores matmuls; ACT trails)
            for kt in range(nkt):
                off = max(0, (kt - 4 * C) * 128)  # cols < off fully masked
                ps = psum.tile([128, 512], F32, tag="mm",
                               name=f"ps{j}_{C}_{kt}")
                nc.tensor.matmul(ps[:, off:512],
                                 kT_sb[:, kt * 128:(kt + 1) * 128],
                                 qT_sb[j][:, qc0 + off: qc0 + 512],
                                 start=True, stop=True)
                pt = ppool.tile([128, 512], BF16, tag="pt",
                                name=f"pt{j}_{C}_{kt}")
                nc.scalar.activation(pt[:, off:512], ps[:, off:512], Exp)
                if kt >= 4 * C:
                    nc.vector.tensor_mul(pt[:, off:off + 128],
                                         pt[:, off:off + 128], tri_sb[:])
                pts.append((kt, off, pt))
                if kt == 1 and carry[0] is not None:
                    carry[0]()
                    carry[0] = None
            # -- rowsum batch (stationary ones-column, negligible ldweights)
            prs = psrs.tile([1, 512], F32, tag="rs", name=f"prs{j}_{C}")
            for kt, off, pt in pts:
                nc.tensor.matmul(prs[:, off:512], onc_sb[:], pt[:, off:512],
                                 start=(kt == 0), stop=(kt == nkt - 1),
                                 skip_group_check=True)
            # -- attn@V batch
            pav = psav.tile([128, 512], F32, tag="av", name=f"pav{j}_{C}")
            for kt, off, pt in pts:
                nc.tensor.matmul(pav[:, off:512],
                                 v_sb[:, kt * 128:(kt + 1) * 128],
                                 pt[:, off:512],
                                 start=(kt == 0), stop=(kt == nkt - 1),
                                 skip_group_check=True)
            carry[0] = make_norm(j, C, pav, prs)
    carry[0]()
    carry[0] = None


def _phase3(nc, tc, wo_sb, o_out, ag_out, agq, agpool, psum, opool):
    def issue_quarter(qq):
        for j in range(3):
            for r in range(N_CORES):
                t = agpool.tile([128, 512], BF16, tag=f"ag{j}_{r}",
                                name=f"ag{j}_{r}_{qq}")
                nc.sync.dma_start(
                    out=t[:],
                    in_=ag_out[j][r * 128:(r + 1) * 128,
                                  qq * 512:(qq + 1) * 512])
                agq[(j, r, qq)] = t

    issue_quarter(2)
    for st in range(NST):
        if st == 6:
            issue_quarter(3)
        qq = st // 4
        c = st % 4
        po = psum.tile([128, 512], F32, tag="mm", name=f"po{st}")
        i = 0
        for j in range(NREP):
            for r in range(N_CORES):
                t = agq[(j, r, qq)]
                nc.tensor.matmul(po[:], t[:, c * 128:(c + 1) * 128],
                                 wo_sb[:, i * 512:(i + 1) * 512],
                                 start=(i == 0), stop=(i == NHT - 1))
                i += 1
        osb = opool.tile([128, 512], F32, tag="o", name=f"o{st}")
        nc.scalar.copy(osb[:], po[:])
        nc.sync.dma_start(out=o_out[st * 128:(st + 1) * 128, :], in_=osb[:])


def prep_inputs(hidden_states, wq, wk, wv, wo, cos, sin, causal_mask=None):
    bf16 = ml_dtypes.bfloat16
    x = np.asarray(hidden_states, np.float32)[0]          # (S, HID)
    xT = np.ascontiguousarray(x.T).astype(bf16)           # (HID, S)
    wq_s = (np.asarray(wq, np.float32) / np.sqrt(HD)).astype(np.float32)
    cos2 = np.asarray(cos, np.float32)[0, 0]              # (S, 64)
    sin2 = np.asarray(sin, np.float32)[0, 0]
    cosT = np.ascontiguousarray(np.concatenate([cos2.T, cos2.T], 0))  # (128, S)
    sinT = np.ascontiguousarray(np.concatenate([sin2.T, sin2.T], 0))
    kl = np.arange(128)[:, None]
    ql = np.arange(128)[None, :]
    triT = (kl <= ql).astype(bf16)                        # allow k <= q
    ones_c = np.ones((128, 1), bf16)
    ones_r = np.ones((1, 128), np.float32)

    # wo reordered to match AllGather row order: row p = j*1024 + r*128 + d
    # corresponds to head (j*8+r), dim d  ->  wo column (j*8+r)*128 + d.
    j_ = np.arange(NREP)[:, None, None]
    r_ = np.arange(N_CORES)[None, :, None]
    d_ = np.arange(HD)[None, None, :]
    col_order = ((j_ * N_CORES + r_) * HD + d_).reshape(-1)
    woT_full = np.ascontiguousarray(
        np.asarray(wo, np.float32)[:, col_order].T).astype(bf16)  # (4096c, 4096hid)

    in_maps = []
    for c in range(N_CORES):
        heads = [jj * N_CORES + c for jj in range(NREP)]
        wq_rows = np.concatenate([wq_s[h * HD:(h + 1) * HD, :] for h in heads], 0)
        wqT_c = np.ascontiguousarray(wq_rows.T).astype(bf16)        # (HID, 512)
        wkT_c = np.ascontiguousarray(
            np.asarray(wk, np.float32)[c * HD:(c + 1) * HD, :].T).astype(bf16)
        wvT_c = np.ascontiguousarray(
            np.asarray(wv, np.float32)[c * HD:(c + 1) * HD, :].T).astype(bf16)
        woT_c = np.ascontiguousarray(woT_full[:, c * 512:(c + 1) * 512])
        in_maps.append(dict(xT=xT, wqT=wqT_c, wkT=wkT_c, wvT=wvT_c, woT=woT_c,
                            cosT=cosT, sinT=sinT, triT=triT,
                            ones_c=ones_c, ones_r=ones_r))
    return in_maps


def postprocess(results):
    out = np.empty((S, HID), np.float32)
    for c in range(N_CORES):
        out[:, c * 512:(c + 1) * 512] = results[c]["o_out"]
    return out[None]


def get_nc():
    if "nc" not in _CACHE:
        _CACHE["nc"] = build_nc()
    return _CACHE["nc"]


def kernel(hidden_states, wq, wk, wv, wo, cos, sin, causal_mask=None):
    nc = get_nc()
    in_maps = prep_inputs(hidden_states, wq, wk, wv, wo, cos, sin, causal_mask)
    res = run_bass_kernel_spmd(nc, in_maps, core_ids=list(range(N_CORES)))
    return postprocess(res.results)


# revision 7
# speedup vs baseline: 1.1618x; 1.1618x over previous
"""Tensor-parallel LlamaAttention (S=2048, HID=4096, NH=32, NKV=8) on 8 trn2 cores.

Sharding: core c owns q heads {c, c+8, c+16, c+24} (head h uses kv head h%8,
so all four share kv head c) and kv head c.  Projections + attention are fully
local; avT (bf16, [128d, 2048s] per head) is AllGathered per head-group, then
each core computes its 512 output columns of o_proj (column-parallel wo).

v2 layout: interleaved startup DMAs (x chunk0 with wq), phase-2 batched into
scores->exp->rowsum->av sub-batches with width-restricted diagonal tiles (no
memsets), deferred normalization, per-chunk AllGather pieces for the last head
group, and arrival-ordered phase-3 accumulation (last group's matmuls last in
each chain) with quartered agt tiles so o_proj overlaps the collectives.

Self-contained: shapes/sharding hardcoded; host does transposes/casts.
"""

import numpy as np
import ml_dtypes

import concourse.bacc as bacc
import concourse.tile as tile
import concourse.mybir as mybir
from concourse.bass_utils import run_bass_kernel_spmd

S = 2048
HID = 4096
NH = 32
NKV = 8
HD = 128
HALF = 64
N_CORES = 8
NREP = NH // NKV  # 4 q heads per core
NHT = HID // 128  # 32 hidden tiles
NST = S // 128    # 16 seq tiles
NSC = S // 512    # 4 seq chunks
BF16 = mybir.dt.bfloat16
F32 = mybir.dt.float32

_CACHE = {}


def build_nc():
    nc = bacc.Bacc("TRN2", target_bir_lowering=False, debug=False,
                   num_devices=N_CORES)

    xT = nc.dram_tensor("xT", [HID, S], BF16, kind="ExternalInput").ap()
    wq = nc.dram_tensor("wqT", [HID, NREP * HD], BF16, kind="ExternalInput").ap()
    wk = nc.dram_tensor("wkT", [HID, HD], BF16, kind="ExternalInput").ap()
    wv = nc.dram_tensor("wvT", [HID, HD], BF16, kind="ExternalInput").ap()
    wo = nc.dram_tensor("woT", [HID, 512], BF16, kind="ExternalInput").ap()
    cosT = nc.dram_tensor("cosT", [HD, S], F32, kind="ExternalInput").ap()
    sinT = nc.dram_tensor("sinT", [HD, S], F32, kind="ExternalInput").ap()
    tri = nc.dram_tensor("triT", [128, 128], BF16, kind="ExternalInput").ap()
    ones_c = nc.dram_tensor("ones_c", [128, 1], BF16, kind="ExternalInput").ap()
    ones_r = nc.dram_tensor("ones_r", [1, 128], F32, kind="ExternalInput").ap()

    o_out = nc.dram_tensor("o_out", [S, 512], F32, kind="ExternalOutput").ap()

    # head groups 0..2: one AllGather each over the full [128, S] avT
    ag_in = [nc.dram_tensor(f"ag_in{j}", [HD, S], BF16).ap() for j in range(3)]
    ag_out = [nc.dram_tensor(f"ag_out{j}", [N_CORES * HD, S], BF16,
                             addr_space="Shared").ap() for j in range(3)]
    # head group 3: per-chunk pieces so the tail exposure is one small piece
    ag3_in = [nc.dram_tensor(f"ag3_in{c}", [HD, 512], BF16).ap()
              for c in range(NSC)]
    ag3_out = [nc.dram_tensor(f"ag3_out{c}", [N_CORES * HD, 512], BF16,
                              addr_space="Shared").ap() for c in range(NSC)]

    with tile.TileContext(nc) as tc:
        _body(nc, tc, xT, wq, wk, wv, wo, cosT, sinT, tri, ones_c, ones_r,
              o_out, ag_in, ag_out, ag3_in, ag3_out)
    nc.compile()
    return nc


def _body(nc, tc, xT, wq, wk, wv, wo, cosT, sinT, tri, ones_c, ones_r,
          o_out, ag_in, ag_out, ag3_in, ag3_out):
    with (
        tc.tile_pool(name="consts", bufs=1) as cpool,
        tc.tile_pool(name="psum", bufs=4, space="PSUM") as psum,
        tc.tile_pool(name="psav", bufs=2, space="PSUM") as psav,
        tc.tile_pool(name="psrs", bufs=2, space="PSUM") as psrs,
    ):
        tri_sb = cpool.tile([128, 128], BF16, tag="tri")
        onc_sb = cpool.tile([128, 1], BF16, tag="onc")
        onr_sb = cpool.tile([1, 128], F32, tag="onr")
        nc.sync.dma_start(out=tri_sb[:], in_=tri[:])
        nc.sync.dma_start(out=onc_sb[:], in_=ones_c[:])
        nc.sync.dma_start(out=onr_sb[:], in_=ones_r[:])

        with tc.tile_pool(name="qkv", bufs=1) as qkvpool:
            qT_sb = [qkvpool.tile([HD, S], BF16, tag=f"qT{j}", name=f"qT{j}")
                     for j in range(NREP)]
            kT_sb = qkvpool.tile([HD, S], BF16, tag="kT")
            v_sb = qkvpool.tile([128, S], BF16, tag="v")  # col block kt = s tile kt

            with (
                tc.tile_pool(name="rconsts", bufs=1) as rcpool,
                tc.tile_pool(name="wproj", bufs=1) as wpool,
                tc.tile_pool(name="xc", bufs=64) as xpool,
                tc.tile_pool(name="rope", bufs=2) as rpool,
            ):
                _phase1(nc, tc, xT, wq, wk, wv, cosT, sinT, qT_sb, kT_sb, v_sb,
                        rcpool, wpool, xpool, rpool, psum)

            with (
                tc.tile_pool(name="probs", bufs=18) as ppool,
                tc.tile_pool(name="avc", bufs=2) as avcpool,
                tc.tile_pool(name="small", bufs=2) as spool,
                tc.tile_pool(name="wo", bufs=1) as wopool,
                tc.tile_pool(name="ag", bufs=3) as agpool,
                tc.tile_pool(name="oout", bufs=2) as opool,
            ):
                # o_proj weights prefetch during phase 2
                wo_sb = wopool.tile([128, NHT * 512], BF16, tag="wo")
                for i in range(NHT):
                    nc.sync.dma_start(out=wo_sb[:, i * 512:(i + 1) * 512],
                                      in_=wo[i * 128:(i + 1) * 128, :])

                agq = {}
                _phase2(nc, tc, qT_sb, kT_sb, v_sb, tri_sb, onc_sb, onr_sb,
                        ag_in, ag_out, ag3_in, ag3_out, agq, agpool,
                        ppool, avcpool, spool, psum, psav, psrs)
                _phase3(nc, tc, wo_sb, o_out, ag_out, agq, agpool, psum, opool)


def _phase1(nc, tc, xT, wq, wk, wv, cosT, sinT, qT_sb, kT_sb, v_sb,
            rcpool, wpool, xpool, rpool, psum):
    cos_sb = rcpool.tile([HD, S], F32, tag="cos")
    sin_sb = rcpool.tile([HD, S], F32, tag="sin")

    wq_sb = wpool.tile([128, NHT * 512], BF16, tag="wq")
    wk_sb = wpool.tile([128, NHT * 128], BF16, tag="wk")
    wv_sb = wpool.tile([128, NHT * 128], BF16, tag="wv")

    # -- DMA issue order tuned for startup: wq and x chunk0 interleaved so the
    #    first q accumulation chains start within a few us, then wk/wv, then
    #    cos/sin (first needed ~30us in, after the first q chains), then the
    #    rest of x (2-chunk prefetch window via pool rotation).
    def _x_dma(cs, h):
        t = xpool.tile([128, 512], BF16, tag="xc", name=f"xc{cs}_{h}")
        nc.sync.dma_start(out=t[:], in_=xT[h * 128:(h + 1) * 128,
                                           cs * 512:(cs + 1) * 512])
        xcs[(cs, h)] = t

    def _cs_dma(cs):
        sc = slice(cs * 512, (cs + 1) * 512)
        nc.sync.dma_start(out=cos_sb[:, sc], in_=cosT[:, sc])
        nc.sync.dma_start(out=sin_sb[:, sc], in_=sinT[:, sc])

    xcs = {}
    for h in range(NHT):
        nc.sync.dma_start(out=wq_sb[:, h * 512:(h + 1) * 512],
                          in_=wq[h * 128:(h + 1) * 128, :])
        _x_dma(0, h)
        if h == 15:
            _cs_dma(0)  # rope chunk0 tables needed ~30us in
    for h in range(NHT):
        nc.sync.dma_start(out=wk_sb[:, h * 128:(h + 1) * 128],
                          in_=wk[h * 128:(h + 1) * 128, :])
        nc.sync.dma_start(out=wv_sb[:, h * 128:(h + 1) * 128],
                          in_=wv[h * 128:(h + 1) * 128, :])
        _x_dma(1, h)
    _cs_dma(1)
    for h in range(NHT):
        _x_dma(2, h)
    _cs_dma(2)
    _cs_dma(3)
    for h in range(NHT):
        _x_dma(3, h)

    for cs in range(NSC):
        sc = slice(cs * 512, (cs + 1) * 512)

        def _rope(dst, pp):
            t1 = rpool.tile([HALF, 512], F32, tag="t1")
            t2 = rpool.tile([HALF, 512], F32, tag="t2")
            nc.vector.tensor_mul(t1[:], pp[0:HALF, :], cos_sb[0:HALF, sc])
            nc.vector.tensor_mul(t2[:], pp[HALF:128, :], sin_sb[0:HALF, sc])
            nc.vector.tensor_sub(dst[0:HALF, sc], t1[:], t2[:])
            t3 = rpool.tile([HALF, 512], F32, tag="t1")
            t4 = rpool.tile([HALF, 512], F32, tag="t2")
            nc.vector.tensor_mul(t3[:], pp[HALF:128, :], cos_sb[HALF:128, sc])
            nc.vector.tensor_mul(t4[:], pp[0:HALF, :], sin_sb[HALF:128, sc])
            nc.vector.tensor_add(dst[HALF:128, sc], t3[:], t4[:])

        for j in range(NREP):
            pq = psum.tile([128, 512], F32, tag="mm")
            for h in range(NHT):
                nc.tensor.matmul(
                    pq[:],
                    wq_sb[:, h * 512 + j * 128: h * 512 + (j + 1) * 128],
                    xcs[(cs, h)][:],
                    start=(h == 0), stop=(h == NHT - 1))
            _rope(qT_sb[j], pq)

        pk = psum.tile([128, 512], F32, tag="mm")
        for h in range(NHT):
            nc.tensor.matmul(pk[:], wk_sb[:, h * 128:(h + 1) * 128],
                             xcs[(cs, h)][:],
                             start=(h == 0), stop=(h == NHT - 1))
        _rope(kT_sb, pk)

        pv = psum.tile([128, 512], F32, tag="mm")
        for tl in range(4):
            for h in range(NHT):
                nc.tensor.matmul(
                    pv[:, tl * 128:(tl + 1) * 128],
                    xcs[(cs, h)][:, tl * 128:(tl + 1) * 128],
                    wv_sb[:, h * 128:(h + 1) * 128],
                    start=(h == 0), stop=(h == NHT - 1))
        nc.scalar.copy(v_sb[:, sc], pv[:])


def _phase2(nc, tc, qT_sb, kT_sb, v_sb, tri_sb, onc_sb, onr_sb,
            ag_in, ag_out, ag3_in, ag3_out, agq, agpool,
            ppool, avcpool, spool, psum, psav, psrs):
    Exp = mybir.ActivationFunctionType.Exp
    carry = [None]

    def make_norm(j, C, pav, prs):
        def f():
            qc = slice(C * 512, (C + 1) * 512)
            rrec = spool.tile([1, 512], F32, tag="rrec", name=f"rrec{j}_{C}")
            nc.vector.reciprocal(rrec[:], prs[:])
            bsb = spool.tile([128, 512], F32, tag="bsb", name=f"bsb{j}_{C}")
            nc.gpsimd.partition_broadcast(bsb[:], rrec[:], channels=128)
            avc = avcpool.tile([128, 512], BF16, tag="avc", name=f"avc{j}_{C}")
            nc.vector.tensor_mul(avc[:], pav[:], bsb[:])
            if j < 3:
                nc.sync.dma_start(out=ag_in[j][:, qc], in_=avc[:])
                if C == NSC - 1:
                    nc.gpsimd.collective_compute(
                        "AllGather", mybir.AluOpType.bypass,
                        replica_groups=[list(range(N_CORES))],
                        ins=[ag_in[j][:]], outs=[ag_out[j][:]])
                    # prefetch quarters 0-1 of this group's agt tiles
                    # separate HWDGE queue (scalar) so these AG-gated loads
                    # can't head-of-line-block later ag_in writes on sync
                    for qq in range(2):
                        for r in range(N_CORES):
                            t = agpool.tile([128, 512], BF16, tag=f"ag{j}_{r}",
                                            name=f"ag{j}_{r}_{qq}")
                            nc.scalar.dma_start(
                                out=t[:],
                                in_=ag_out[j][r * 128:(r + 1) * 128,
                                              qq * 512:(qq + 1) * 512])
                            agq[(j, r, qq)] = t
            else:
                nc.sync.dma_start(out=ag3_in[C][:], in_=avc[:])
                nc.gpsimd.collective_compute(
                    "AllGather", mybir.AluOpType.bypass,
                    replica_groups=[list(range(N_CORES))],
                    ins=[ag3_in[C][:]], outs=[ag3_out[C][:]])
                for r in range(N_CORES):
                    t = agpool.tile([128, 512], BF16, tag=f"ag3_{r}",
                                    name=f"ag3_{r}_{C}")
                    nc.scalar.dma_start(
                        out=t[:], in_=ag3_out[C][r * 128:(r + 1) * 128, :])
                    agq[(3, r, C)] = t
        return f

    for j in range(NREP):
        for C in range(NSC):
            qc0 = C * 512
            nkt = 4 * C + 4
            pts = []
            # -- scores + exp batch (PE: contiguous scores matmuls; ACT trails)
            for kt in range(nkt):
                off = max(0, (kt - 4 * C) * 128)  # cols < off fully masked
                ps = psum.tile([128, 512], F32, tag="mm",
                               name=f"ps{j}_{C}_{kt}")
                nc.tensor.matmul(ps[:, off:512],
                                 kT_sb[:, kt * 128:(kt + 1) * 128],
                                 qT_sb[j][:, qc0 + off: qc0 + 512],
                                 start=True, stop=True)
                pt = ppool.tile([128, 512], BF16, tag="pt",
                                name=f"pt{j}_{C}_{kt}")
                nc.scalar.activation(pt[:, off:512], ps[:, off:512], Exp)
                if kt >= 4 * C:
                    nc.vector.tensor_mul(pt[:, off:off + 128],
                                         pt[:, off:off + 128], tri_sb[:])
                pts.append((kt, off, pt))
                if kt == 1 and carry[0] is not None:
                    carry[0]()
                    carry[0] = None
            # -- rowsum batch (stationary ones-column, negligible ldweights)
            prs = psrs.tile([1, 512], F32, tag="rs", name=f"prs{j}_{C}")
            for kt, off, pt in pts:
                nc.tensor.matmul(prs[:, off:512], onc_sb[:], pt[:, off:512],
                                 start=(kt == 0), stop=(kt == nkt - 1),
                                 skip_group_check=True)
            # -- attn@V batch
            pav = psav.tile([128, 512], F32, tag="av", name=f"pav{j}_{C}")
            for kt, off, pt in pts:
                nc.tensor.matmul(pav[:, off:512],
                                 v_sb[:, kt * 128:(kt + 1) * 128],
                                 pt[:, off:512],
                                 start=(kt == 0), stop=(kt == nkt - 1),
                                 skip_group_check=True)
            carry[0] = make_norm(j, C, pav, prs)
    carry[0]()
    carry[0] = None


def _phase3(nc, tc, wo_sb, o_out, ag_out, agq, agpool, psum, opool):
    def issue_quarter(qq):
        for j in range(3):
            for r in range(N_CORES):
                t = agpool.tile([128, 512], BF16, tag=f"ag{j}_{r}",
                                name=f"ag{j}_{r}_{qq}")
                nc.scalar.dma_start(
                    out=t[:],
                    in_=ag_out[j][r * 128:(r + 1) * 128,
                                  qq * 512:(qq + 1) * 512])
                agq[(j, r, qq)] = t

    issue_quarter(2)
    for st in range(NST):
        if st == 6:
            issue_quarter(3)
        qq = st // 4
        c = st % 4
        po = psum.tile([128, 512], F32, tag="mm", name=f"po{st}")
        i = 0
        for j in range(NREP):
            for r in range(N_CORES):
                t = agq[(j, r, qq)]
                nc.tensor.matmul(po[:], t[:, c * 128:(c + 1) * 128],
                                 wo_sb[:, i * 512:(i + 1) * 512],
                                 start=(i == 0), stop=(i == NHT - 1))
                i += 1
        osb = opool.tile([128, 512], F32, tag="o", name=f"o{st}")
        nc.scalar.copy(osb[:], po[:])
        nc.sync.dma_start(out=o_out[st * 128:(st + 1) * 128, :], in_=osb[:])


def prep_inputs(hidden_states, wq, wk, wv, wo, cos, sin, causal_mask=None):
    bf16 = ml_dtypes.bfloat16
    x = np.asarray(hidden_states, np.float32)[0]          # (S, HID)
    xT = np.ascontiguousarray(x.T).astype(bf16)           # (HID, S)
    wq_s = (np.asarray(wq, np.float32) / np.sqrt(HD)).astype(np.float32)
    cos2 = np.asarray(cos, np.float32)[0, 0]              # (S, 64)
    sin2 = np.asarray(sin, np.float32)[0, 0]
    cosT = np.ascontiguousarray(np.concatenate([cos2.T, cos2.T], 0))  # (128, S)
    sinT = np.ascontiguousarray(np.concatenate([sin2.T, sin2.T], 0))
    kl = np.arange(128)[:, None]
    ql = np.arange(128)[None, :]
    triT = (kl <= ql).astype(bf16)                        # allow k <= q
    ones_c = np.ones((128, 1), bf16)
    ones_r = np.ones((1, 128), np.float32)

    # wo reordered to match AllGather row order: row p = j*1024 + r*128 + d
    # corresponds to head (j*8+r), dim d  ->  wo column (j*8+r)*128 + d.
    j_ = np.arange(NREP)[:, None, None]
    r_ = np.arange(N_CORES)[None, :, None]
    d_ = np.arange(HD)[None, None, :]
    col_order = ((j_ * N_CORES + r_) * HD + d_).reshape(-1)
    woT_full = np.ascontiguousarray(
        np.asarray(wo, np.float32)[:, col_order].T).astype(bf16)  # (4096c, 4096hid)

    in_maps = []
    for c in range(N_CORES):
        heads = [jj * N_CORES + c for jj in range(NREP)]
        wq_rows = np.concatenate([wq_s[h * HD:(h + 1) * HD, :] for h in heads], 0)
        wqT_c = np.ascontiguousarray(wq_rows.T).astype(bf16)        # (HID, 512)
        wkT_c = np.ascontiguousarray(
            np.asarray(wk, np.float32)[c * HD:(c + 1) * HD, :].T).astype(bf16)
        wvT_c = np.ascontiguousarray(
            np.asarray(wv, np.float32)[c * HD:(c + 1) * HD, :].T).astype(bf16)
        woT_c = np.ascontiguousarray(woT_full[:, c * 512:(c + 1) * 512])
        in_maps.append(dict(xT=xT, wqT=wqT_c, wkT=wkT_c, wvT=wvT_c, woT=woT_c,
                            cosT=cosT, sinT=sinT, triT=triT,
                            ones_c=ones_c, ones_r=ones_r))
    return in_maps


def postprocess(results):
    out = np.empty((S, HID), np.float32)
    for c in range(N_CORES):
        out[:, c * 512:(c + 1) * 512] = results[c]["o_out"]
    return out[None]


def get_nc():
    if "nc" not in _CACHE:
        _CACHE["nc"] = build_nc()
    return _CACHE["nc"]


def kernel(hidden_states, wq, wk, wv, wo, cos, sin, causal_mask=None):
    nc = get_nc()
    in_maps = prep_inputs(hidden_states, wq, wk, wv, wo, cos, sin, causal_mask)
    res = run_bass_kernel_spmd(nc, in_maps, core_ids=list(range(N_CORES)))
    return postprocess(res.results)


# revision 12
# speedup vs baseline: 1.2764x; 1.0986x over previous
"""Tensor-parallel LlamaAttention (S=2048, HID=4096, NH=32, NKV=8) on 8 trn2 cores.

Sharding: core c owns q heads {c, c+8, c+16, c+24} (head h uses kv head h%8,
so all four share kv head c) and kv head c.  Projections + attention are fully
local; avT (bf16, [128d, 2048s] per head) is AllGathered per head-group, then
each core computes its 512 output columns of o_proj (column-parallel wo).

v2 layout: interleaved startup DMAs (x chunk0 with wq), phase-2 batched into
scores->exp->rowsum->av sub-batches with width-restricted diagonal tiles (no
memsets), deferred normalization, per-chunk AllGather pieces for the last head
group, and arrival-ordered phase-3 accumulation (last group's matmuls last in
each chain) with quartered agt tiles so o_proj overlaps the collectives.

Self-contained: shapes/sharding hardcoded; host does transposes/casts.
"""

import numpy as np
import ml_dtypes

import concourse.bacc as bacc
import concourse.tile as tile
import concourse.mybir as mybir
from concourse.bass_utils import run_bass_kernel_spmd

S = 2048
HID = 4096
NH = 32
NKV = 8
HD = 128
HALF = 64
N_CORES = 8
NREP = NH // NKV  # 4 q heads per core
NHT = HID // 128  # 32 hidden tiles
NST = S // 128    # 16 seq tiles
NSC = S // 512    # 4 seq chunks
BF16 = mybir.dt.bfloat16
F32 = mybir.dt.float32

_CACHE = {}


def build_nc():
    nc = bacc.Bacc("TRN2", target_bir_lowering=False, debug=False,
                   num_devices=N_CORES)

    xT = nc.dram_tensor("xT", [HID, S], BF16, kind="ExternalInput").ap()
    wq = nc.dram_tensor("wqT", [HID, NREP * HD], BF16, kind="ExternalInput").ap()
    wk = nc.dram_tensor("wkT", [HID, HD], BF16, kind="ExternalInput").ap()
    wv = nc.dram_tensor("wvT", [HID, HD], BF16, kind="ExternalInput").ap()
    wo = nc.dram_tensor("woT", [HID, 512], BF16, kind="ExternalInput").ap()
    cosT = nc.dram_tensor("cosT", [HD, S], F32, kind="ExternalInput").ap()
    sinT = nc.dram_tensor("sinT", [HD, S], F32, kind="ExternalInput").ap()
    tri = nc.dram_tensor("triT", [128, 128], BF16, kind="ExternalInput").ap()
    ones_c = nc.dram_tensor("ones_c", [128, 1], BF16, kind="ExternalInput").ap()
    ones_r = nc.dram_tensor("ones_r", [1, 128], F32, kind="ExternalInput").ap()

    o_out = nc.dram_tensor("o_out", [S, 512], F32, kind="ExternalOutput").ap()

    # head groups 0..2: one AllGather each over the full [128, S] avT
    ag_in = [nc.dram_tensor(f"ag_in{j}", [HD, S], BF16).ap() for j in range(3)]
    ag_out = [nc.dram_tensor(f"ag_out{j}", [N_CORES * HD, S], BF16,
                             addr_space="Shared").ap() for j in range(3)]
    # head group 3: per-chunk pieces so the tail exposure is one small piece
    ag3_in = [nc.dram_tensor(f"ag3_in{c}", [HD, 512], BF16).ap()
              for c in range(NSC)]
    ag3_out = [nc.dram_tensor(f"ag3_out{c}", [N_CORES * HD, 512], BF16,
                              addr_space="Shared").ap() for c in range(NSC)]

    with tile.TileContext(nc) as tc:
        _body(nc, tc, xT, wq, wk, wv, wo, cosT, sinT, tri, ones_c, ones_r,
              o_out, ag_in, ag_out, ag3_in, ag3_out)
    nc.compile()
    return nc


def _body(nc, tc, xT, wq, wk, wv, wo, cosT, sinT, tri, ones_c, ones_r,
          o_out, ag_in, ag_out, ag3_in, ag3_out):
    with (
        tc.tile_pool(name="consts", bufs=1) as cpool,
        tc.tile_pool(name="psum", bufs=4, space="PSUM") as psum,
        tc.tile_pool(name="psav", bufs=2, space="PSUM") as psav,
        tc.tile_pool(name="psrs", bufs=2, space="PSUM") as psrs,
    ):
        tri_sb = cpool.tile([128, 128], BF16, tag="tri")
        ones_sb = cpool.tile([128, 128], BF16, tag="ones")
        nc.sync.dma_start(out=tri_sb[:], in_=tri[:])
        nc.vector.memset(ones_sb[:], 1.0)

        with tc.tile_pool(name="qkv", bufs=1) as qkvpool:
            qT_sb = [qkvpool.tile([HD, S], BF16, tag=f"qT{j}", name=f"qT{j}")
                     for j in range(NREP)]
            kT_sb = qkvpool.tile([HD, S], BF16, tag="kT")
            v_sb = qkvpool.tile([128, S], BF16, tag="v")  # col block kt = s tile kt

            with (
                tc.tile_pool(name="rconsts", bufs=1) as rcpool,
                tc.tile_pool(name="wproj", bufs=1) as wpool,
                tc.tile_pool(name="xc", bufs=64) as xpool,
                tc.tile_pool(name="rope", bufs=2) as rpool,
            ):
                _phase1(nc, tc, xT, wq, wk, wv, cosT, sinT, qT_sb, kT_sb, v_sb,
                        rcpool, wpool, xpool, rpool, psum)

            with (
                tc.tile_pool(name="probs", bufs=18) as ppool,
                tc.tile_pool(name="avc", bufs=2) as avcpool,
                tc.tile_pool(name="small", bufs=2) as spool,
                tc.tile_pool(name="wo", bufs=1) as wopool,
                tc.tile_pool(name="ag", bufs=3) as agpool,
                tc.tile_pool(name="oout", bufs=2) as opool,
            ):
                # o_proj weights prefetch during phase 2
                wo_sb = wopool.tile([128, NHT * 512], BF16, tag="wo")
                for i in range(NHT):
                    nc.sync.dma_start(out=wo_sb[:, i * 512:(i + 1) * 512],
                                      in_=wo[i * 128:(i + 1) * 128, :])

                agq = {}
                _phase2(nc, tc, qT_sb, kT_sb, v_sb, tri_sb, ones_sb,
                        ag_in, ag_out, ag3_in, ag3_out, agq, agpool,
                        ppool, avcpool, spool, psum, psav, psrs)
                _phase3(nc, tc, wo_sb, o_out, ag_out, agq, agpool, psum, opool)


def _phase1(nc, tc, xT, wq, wk, wv, cosT, sinT, qT_sb, kT_sb, v_sb,
            rcpool, wpool, xpool, rpool, psum):
    cos_sb = rcpool.tile([HD, S], F32, tag="cos")
    sin_sb = rcpool.tile([HD, S], F32, tag="sin")

    wq_sb = wpool.tile([128, NHT * 512], BF16, tag="wq")
    wk_sb = wpool.tile([128, NHT * 128], BF16, tag="wk")
    wv_sb = wpool.tile([128, NHT * 128], BF16, tag="wv")

    # -- DMA issue order tuned for startup: wq and x chunk0 interleaved so the
    #    first q accumulation chains start within a few us, then wk/wv, then
    #    cos/sin (first needed ~30us in, after the first q chains), then the
    #    rest of x (2-chunk prefetch window via pool rotation).
    def _x_dma(cs, h):
        t = xpool.tile([128, 512], BF16, tag="xc", name=f"xc{cs}_{h}")
        nc.sync.dma_start(out=t[:], in_=xT[h * 128:(h + 1) * 128,
                                           cs * 512:(cs + 1) * 512])
        xcs[(cs, h)] = t

    def _cs_dma(cs):
        sc = slice(cs * 512, (cs + 1) * 512)
        nc.sync.dma_start(out=cos_sb[:, sc], in_=cosT[:, sc])
        nc.sync.dma_start(out=sin_sb[:, sc], in_=sinT[:, sc])

    xcs = {}
    for h in range(NHT):
        nc.sync.dma_start(out=wq_sb[:, h * 512:(h + 1) * 512],
                          in_=wq[h * 128:(h + 1) * 128, :])
        _x_dma(0, h)
        nc.sync.dma_start(out=wk_sb[:, h * 128:(h + 1) * 128],
                          in_=wk[h * 128:(h + 1) * 128, :])
        nc.sync.dma_start(out=wv_sb[:, h * 128:(h + 1) * 128],
                          in_=wv[h * 128:(h + 1) * 128, :])
        if h == 12:
            _cs_dma(0)  # rope chunk0 tables needed ~35us in
    for h in range(NHT):
        _x_dma(1, h)
    _cs_dma(1)
    for h in range(NHT):
        _x_dma(2, h)
    _cs_dma(2)
    _cs_dma(3)
    for h in range(NHT):
        _x_dma(3, h)

    for cs in range(NSC):
        sc = slice(cs * 512, (cs + 1) * 512)

        def _rope(dst, pp):
            t1 = rpool.tile([HALF, 512], F32, tag="t1")
            t2 = rpool.tile([HALF, 512], F32, tag="t2")
            nc.vector.tensor_mul(t1[:], pp[0:HALF, :], cos_sb[0:HALF, sc])
            nc.vector.tensor_mul(t2[:], pp[HALF:128, :], sin_sb[0:HALF, sc])
            nc.vector.tensor_sub(dst[0:HALF, sc], t1[:], t2[:])
            t3 = rpool.tile([HALF, 512], F32, tag="t1")
            t4 = rpool.tile([HALF, 512], F32, tag="t2")
            nc.vector.tensor_mul(t3[:], pp[HALF:128, :], cos_sb[HALF:128, sc])
            nc.vector.tensor_mul(t4[:], pp[0:HALF, :], sin_sb[HALF:128, sc])
            nc.vector.tensor_add(dst[HALF:128, sc], t3[:], t4[:])

        for j in range(NREP):
            pq = psum.tile([128, 512], F32, tag="mm")
            for h in range(NHT):
                nc.tensor.matmul(
                    pq[:],
                    wq_sb[:, h * 512 + j * 128: h * 512 + (j + 1) * 128],
                    xcs[(cs, h)][:],
                    start=(h == 0), stop=(h == NHT - 1))
            _rope(qT_sb[j], pq)

        pk = psum.tile([128, 512], F32, tag="mm")
        for h in range(NHT):
            nc.tensor.matmul(pk[:], wk_sb[:, h * 128:(h + 1) * 128],
                             xcs[(cs, h)][:],
                             start=(h == 0), stop=(h == NHT - 1))
        _rope(kT_sb, pk)

        pv = psum.tile([128, 512], F32, tag="mm")
        for tl in range(4):
            for h in range(NHT):
                nc.tensor.matmul(
                    pv[:, tl * 128:(tl + 1) * 128],
                    xcs[(cs, h)][:, tl * 128:(tl + 1) * 128],
                    wv_sb[:, h * 128:(h + 1) * 128],
                    start=(h == 0), stop=(h == NHT - 1))
        nc.scalar.copy(v_sb[:, sc], pv[:])


def _phase2(nc, tc, qT_sb, kT_sb, v_sb, tri_sb, ones_sb,
            ag_in, ag_out, ag3_in, ag3_out, agq, agpool,
            ppool, avcpool, spool, psum, psav, psrs):
    Exp = mybir.ActivationFunctionType.Exp
    carry = [None]

    def issue_agt(j, qq):
        # sync queue, at loop points where AG_j is (nearly) complete, so the
        # in-order queue never blocks anything urgent behind these
        for r in range(N_CORES):
            t = agpool.tile([128, 512], BF16, tag=f"ag{j}_{r}",
                            name=f"ag{j}_{r}_{qq}")
            nc.sync.dma_start(out=t[:],
                              in_=ag_out[j][r * 128:(r + 1) * 128,
                                            qq * 512:(qq + 1) * 512])
            agq[(j, r, qq)] = t

    def make_norm(j, C, pav, prs):
        def f():
            qc = slice(C * 512, (C + 1) * 512)
            # prs already has the rowsum broadcast on all 128 partitions
            bsb = spool.tile([128, 512], F32, tag="bsb", name=f"bsb{j}_{C}")
            nc.vector.reciprocal(bsb[:], prs[:])
            avc = avcpool.tile([128, 512], BF16, tag="avc", name=f"avc{j}_{C}")
            nc.vector.tensor_mul(avc[:], pav[:], bsb[:])
            if j < 3:
                nc.gpsimd.dma_start(out=ag_in[j][:, qc], in_=avc[:])
                if C == NSC - 1:
                    nc.gpsimd.collective_compute(
                        "AllGather", mybir.AluOpType.bypass,
                        replica_groups=[list(range(N_CORES))],
                        ins=[ag_in[j][:]], outs=[ag_out[j][:]])
            else:
                nc.gpsimd.dma_start(out=ag3_in[C][:], in_=avc[:])
                nc.gpsimd.collective_compute(
                    "AllGather", mybir.AluOpType.bypass,
                    replica_groups=[list(range(N_CORES))],
                    ins=[ag3_in[C][:]], outs=[ag3_out[C][:]])
                for r in range(N_CORES):
                    t = agpool.tile([128, 512], BF16, tag=f"ag3_{r}",
                                    name=f"ag3_{r}_{C}")
                    nc.sync.dma_start(
                        out=t[:], in_=ag3_out[C][r * 128:(r + 1) * 128, :])
                    agq[(3, r, C)] = t
        return f

    # agt prefetch schedule: (j, C) loop position -> quarters safe to issue
    prefetch = {(2, 0): [(0, 0)], (2, 2): [(0, 1)],
                (3, 0): [(1, 0)], (3, 1): [(1, 1)],
                (3, 2): [(2, 0)], (3, 3): [(2, 1)]}

    for j in range(NREP):
        for C in range(NSC):
            for (jj, qq) in prefetch.get((j, C), ()):
                issue_agt(jj, qq)
            qc0 = C * 512
            nkt = 4 * C + 4
            prs = psrs.tile([128, 512], F32, tag="rs", name=f"prs{j}_{C}")
            pav = psav.tile([128, 512], F32, tag="av", name=f"pav{j}_{C}")
            pend = []

            def drain_one():
                kt2, off2, pt2 = pend.pop(0)
                nc.tensor.matmul(prs[:, off2:512], ones_sb[:],
                                 pt2[:, off2:512],
                                 start=(kt2 == 0), stop=(kt2 == nkt - 1),
                                 skip_group_check=True)
                nc.tensor.matmul(pav[:, off2:512],
                                 v_sb[:, kt2 * 128:(kt2 + 1) * 128],
                                 pt2[:, off2:512],
                                 start=(kt2 == 0), stop=(kt2 == nkt - 1),
                                 skip_group_check=True)

            for kt in range(nkt):
                off = max(0, (kt - 4 * C) * 128)  # cols < off fully masked
                ps = psum.tile([128, 512], F32, tag="mm",
                               name=f"ps{j}_{C}_{kt}")
                nc.tensor.matmul(ps[:, off:512],
                                 kT_sb[:, kt * 128:(kt + 1) * 128],
                                 qT_sb[j][:, qc0 + off: qc0 + 512],
                                 start=True, stop=True)
                pt = ppool.tile([128, 512], BF16, tag="pt",
                                name=f"pt{j}_{C}_{kt}")
                nc.scalar.activation(pt[:, off:512], ps[:, off:512], Exp)
                if kt >= 4 * C:
                    nc.vector.tensor_mul(pt[:, off:off + 128],
                                         pt[:, off:off + 128], tri_sb[:])
                pend.append((kt, off, pt))
                if kt == 1 and carry[0] is not None:
                    carry[0]()
                    carry[0] = None
                if len(pend) > 3:
                    drain_one()
            while pend:
                drain_one()
            carry[0] = make_norm(j, C, pav, prs)
    carry[0]()
    carry[0] = None


def _phase3(nc, tc, wo_sb, o_out, ag_out, agq, agpool, psum, opool):
    def issue_quarter(qq):
        for j in range(3):
            for r in range(N_CORES):
                t = agpool.tile([128, 512], BF16, tag=f"ag{j}_{r}",
                                name=f"ag{j}_{r}_{qq}")
                nc.sync.dma_start(
                    out=t[:],
                    in_=ag_out[j][r * 128:(r + 1) * 128,
                                  qq * 512:(qq + 1) * 512])
                agq[(j, r, qq)] = t

    issue_quarter(2)
    for st in range(NST):
        if st == 6:
            issue_quarter(3)
        qq = st // 4
        c = st % 4
        po = psum.tile([128, 512], F32, tag="mm", name=f"po{st}")
        i = 0
        for j in range(NREP):
            for r in range(N_CORES):
                t = agq[(j, r, qq)]
                nc.tensor.matmul(po[:], t[:, c * 128:(c + 1) * 128],
                                 wo_sb[:, i * 512:(i + 1) * 512],
                                 start=(i == 0), stop=(i == NHT - 1))
                i += 1
        osb = opool.tile([128, 512], F32, tag="o", name=f"o{st}")
        nc.scalar.copy(osb[:], po[:])
        nc.sync.dma_start(out=o_out[st * 128:(st + 1) * 128, :], in_=osb[:])


def prep_inputs(hidden_states, wq, wk, wv, wo, cos, sin, causal_mask=None):
    bf16 = ml_dtypes.bfloat16
    x = np.asarray(hidden_states, np.float32)[0]          # (S, HID)
    xT = np.ascontiguousarray(x.T).astype(bf16)           # (HID, S)
    wq_s = (np.asarray(wq, np.float32) / np.sqrt(HD)).astype(np.float32)
    cos2 = np.asarray(cos, np.float32)[0, 0]              # (S, 64)
    sin2 = np.asarray(sin, np.float32)[0, 0]
    cosT = np.ascontiguousarray(np.concatenate([cos2.T, cos2.T], 0))  # (128, S)
    sinT = np.ascontiguousarray(np.concatenate([sin2.T, sin2.T], 0))
    kl = np.arange(128)[:, None]
    ql = np.arange(128)[None, :]
    triT = (kl <= ql).astype(bf16)                        # allow k <= q
    ones_c = np.ones((128, 1), bf16)
    ones_r = np.ones((1, 128), np.float32)

    # wo reordered to match AllGather row order: row p = j*1024 + r*128 + d
    # corresponds to head (j*8+r), dim d  ->  wo column (j*8+r)*128 + d.
    j_ = np.arange(NREP)[:, None, None]
    r_ = np.arange(N_CORES)[None, :, None]
    d_ = np.arange(HD)[None, None, :]
    col_order = ((j_ * N_CORES + r_) * HD + d_).reshape(-1)
    woT_full = np.ascontiguousarray(
        np.asarray(wo, np.float32)[:, col_order].T).astype(bf16)  # (4096c, 4096hid)

    in_maps = []
    for c in range(N_CORES):
        heads = [jj * N_CORES + c for jj in range(NREP)]
        wq_rows = np.concatenate([wq_s[h * HD:(h + 1) * HD, :] for h in heads], 0)
        wqT_c = np.ascontiguousarray(wq_rows.T).astype(bf16)        # (HID, 512)
        wkT_c = np.ascontiguousarray(
            np.asarray(wk, np.float32)[c * HD:(c + 1) * HD, :].T).astype(bf16)
        wvT_c = np.ascontiguousarray(
            np.asarray(wv, np.float32)[c * HD:(c + 1) * HD, :].T).astype(bf16)
        woT_c = np.ascontiguousarray(woT_full[:, c * 512:(c + 1) * 512])
        in_maps.append(dict(xT=xT, wqT=wqT_c, wkT=wkT_c, wvT=wvT_c, woT=woT_c,
                            cosT=cosT, sinT=sinT, triT=triT,
                            ones_c=ones_c, ones_r=ones_r))
    return in_maps


def postprocess(results):
    out = np.empty((S, HID), np.float32)
    for c in range(N_CORES):
        out[:, c * 512:(c + 1) * 512] = results[c]["o_out"]
    return out[None]


def get_nc():
    if "nc" not in _CACHE:
        _CACHE["nc"] = build_nc()
    return _CACHE["nc"]


def kernel(hidden_states, wq, wk, wv, wo, cos, sin, causal_mask=None):
    nc = get_nc()
    in_maps = prep_inputs(hidden_states, wq, wk, wv, wo, cos, sin, causal_mask)
    res = run_bass_kernel_spmd(nc, in_maps, core_ids=list(range(N_CORES)))
    return postprocess(res.results)


# revision 16
# speedup vs baseline: 1.2885x; 1.0095x over previous
"""Tensor-parallel LlamaAttention (S=2048, HID=4096, NH=32, NKV=8) on 8 trn2 cores.

Sharding: core c owns q heads {c, c+8, c+16, c+24} (head h uses kv head h%8,
so all four share kv head c) and kv head c.  Projections + attention are fully
local; avT (bf16, [128d, 2048s] per head) is AllGathered per head-group, then
each core computes its 512 output columns of o_proj (column-parallel wo).

v2 layout: interleaved startup DMAs (x chunk0 with wq), phase-2 batched into
scores->exp->rowsum->av sub-batches with width-restricted diagonal tiles (no
memsets), deferred normalization, per-chunk AllGather pieces for the last head
group, and arrival-ordered phase-3 accumulation (last group's matmuls last in
each chain) with quartered agt tiles so o_proj overlaps the collectives.

Self-contained: shapes/sharding hardcoded; host does transposes/casts.
"""

import numpy as np
import ml_dtypes

import concourse.bacc as bacc
import concourse.tile as tile
import concourse.mybir as mybir
from concourse.bass_utils import run_bass_kernel_spmd

S = 2048
HID = 4096
NH = 32
NKV = 8
HD = 128
HALF = 64
N_CORES = 8
NREP = NH // NKV  # 4 q heads per core
NHT = HID // 128  # 32 hidden tiles
NST = S // 128    # 16 seq tiles
NSC = S // 512    # 4 seq chunks
BF16 = mybir.dt.bfloat16
F32 = mybir.dt.float32

_CACHE = {}


def build_nc():
    nc = bacc.Bacc("TRN2", target_bir_lowering=False, debug=False,
                   num_devices=N_CORES)

    xT = nc.dram_tensor("xT", [HID, S], BF16, kind="ExternalInput").ap()
    wq = nc.dram_tensor("wqT", [HID, NREP * HD], BF16, kind="ExternalInput").ap()
    wk = nc.dram_tensor("wkT", [HID, HD], BF16, kind="ExternalInput").ap()
    wv = nc.dram_tensor("wvT", [HID, HD], BF16, kind="ExternalInput").ap()
    wo = nc.dram_tensor("woT", [HID, 512], BF16, kind="ExternalInput").ap()
    cosT = nc.dram_tensor("cosT", [HD, S], F32, kind="ExternalInput").ap()
    sinT = nc.dram_tensor("sinT", [HD, S], F32, kind="ExternalInput").ap()
    tri = nc.dram_tensor("triT", [128, 128], BF16, kind="ExternalInput").ap()
    ones_c = nc.dram_tensor("ones_c", [128, 1], BF16, kind="ExternalInput").ap()
    ones_r = nc.dram_tensor("ones_r", [1, 128], F32, kind="ExternalInput").ap()

    o_out = nc.dram_tensor("o_out", [S, 512], F32, kind="ExternalOutput").ap()

    # head groups 0..2: one AllGather each over the full [128, S] avT
    ag_in = [nc.dram_tensor(f"ag_in{j}", [HD, S], BF16).ap() for j in range(3)]
    ag_out = [nc.dram_tensor(f"ag_out{j}", [N_CORES * HD, S], BF16,
                             addr_space="Shared").ap() for j in range(3)]
    # head group 3: per-chunk pieces so the tail exposure is one small piece
    ag3_in = [nc.dram_tensor(f"ag3_in{c}", [HD, 512], BF16).ap()
              for c in range(NSC)]
    ag3_out = [nc.dram_tensor(f"ag3_out{c}", [N_CORES * HD, 512], BF16,
                              addr_space="Shared").ap() for c in range(NSC)]
    # tiny barrier collective: aligns cores at phase-2 start so AG_0's
    # rendezvous (and the whole serial CC pipeline behind it) isn't delayed
    # by per-core drift accumulated during phase 1
    bar_in = nc.dram_tensor("bar_in", [HD, 16], BF16).ap()
    bar_out = nc.dram_tensor("bar_out", [N_CORES * HD, 16], BF16,
                             addr_space="Shared").ap()

    with tile.TileContext(nc) as tc:
        _body(nc, tc, xT, wq, wk, wv, wo, cosT, sinT, tri, ones_c, ones_r,
              o_out, ag_in, ag_out, ag3_in, ag3_out, bar_in, bar_out)
    nc.compile()
    return nc


def _body(nc, tc, xT, wq, wk, wv, wo, cosT, sinT, tri, ones_c, ones_r,
          o_out, ag_in, ag_out, ag3_in, ag3_out, bar_in, bar_out):
    with (
        tc.tile_pool(name="consts", bufs=1) as cpool,
        tc.tile_pool(name="psum", bufs=4, space="PSUM") as psum,
        tc.tile_pool(name="psav", bufs=2, space="PSUM") as psav,
        tc.tile_pool(name="psrs", bufs=2, space="PSUM") as psrs,
    ):
        tri_sb = cpool.tile([128, 128], BF16, tag="tri")
        ones_sb = cpool.tile([128, 128], BF16, tag="ones")
        nc.sync.dma_start(out=tri_sb[:], in_=tri[:])
        nc.vector.memset(ones_sb[:], 1.0)

        with tc.tile_pool(name="qkv", bufs=1) as qkvpool:
            qT_sb = [qkvpool.tile([HD, S], BF16, tag=f"qT{j}", name=f"qT{j}")
                     for j in range(NREP)]
            kT_sb = qkvpool.tile([HD, S], BF16, tag="kT")
            v_sb = qkvpool.tile([128, S], BF16, tag="v")  # col block kt = s tile kt

            with (
                tc.tile_pool(name="rconsts", bufs=1) as rcpool,
                tc.tile_pool(name="wproj", bufs=1) as wpool,
                tc.tile_pool(name="xc", bufs=64) as xpool,
                tc.tile_pool(name="rope", bufs=2) as rpool,
            ):
                _phase1(nc, tc, xT, wq, wk, wv, cosT, sinT, qT_sb, kT_sb, v_sb,
                        rcpool, wpool, xpool, rpool, psum)

            with (
                tc.tile_pool(name="probs", bufs=18) as ppool,
                tc.tile_pool(name="avc", bufs=4) as avcpool,
                tc.tile_pool(name="small", bufs=4) as spool,
                tc.tile_pool(name="wo", bufs=1) as wopool,
                tc.tile_pool(name="ag", bufs=3) as agpool,
                tc.tile_pool(name="oout", bufs=2) as opool,
            ):
                # core-alignment barrier: bar_in copy depends on the tail of
                # phase-1 kT, so the collective fires when phase 1 completes
                nc.gpsimd.dma_start(out=bar_in[:], in_=kT_sb[:, S - 16:S])
                nc.gpsimd.collective_compute(
                    "AllGather", mybir.AluOpType.bypass,
                    replica_groups=[list(range(N_CORES))],
                    ins=[bar_in[:]], outs=[bar_out[:]])
                # o_proj weights prefetch during phase 2
                wo_sb = wopool.tile([128, NHT * 512], BF16, tag="wo")
                for i in range(NHT):
                    nc.sync.dma_start(out=wo_sb[:, i * 512:(i + 1) * 512],
                                      in_=wo[i * 128:(i + 1) * 128, :])

                agq = {}
                _phase2(nc, tc, qT_sb, kT_sb, v_sb, tri_sb, ones_sb,
                        ag_in, ag_out, ag3_in, ag3_out, agq, agpool,
                        ppool, avcpool, spool, psum, psav, psrs)
                _phase3(nc, tc, wo_sb, o_out, ag_out, agq, agpool, psum, opool)


def _phase1(nc, tc, xT, wq, wk, wv, cosT, sinT, qT_sb, kT_sb, v_sb,
            rcpool, wpool, xpool, rpool, psum):
    cos_sb = rcpool.tile([HD, S], F32, tag="cos")
    sin_sb = rcpool.tile([HD, S], F32, tag="sin")

    wq_sb = wpool.tile([128, NHT * 512], BF16, tag="wq")
    wk_sb = wpool.tile([128, NHT * 128], BF16, tag="wk")
    wv_sb = wpool.tile([128, NHT * 128], BF16, tag="wv")

    # -- DMA issue order tuned for startup: wq and x chunk0 interleaved so the
    #    first q accumulation chains start within a few us, then wk/wv, then
    #    cos/sin (first needed ~30us in, after the first q chains), then the
    #    rest of x (2-chunk prefetch window via pool rotation).
    def _x_dma(cs, h):
        t = xpool.tile([128, 512], BF16, tag="xc", name=f"xc{cs}_{h}")
        nc.sync.dma_start(out=t[:], in_=xT[h * 128:(h + 1) * 128,
                                           cs * 512:(cs + 1) * 512])
        xcs[(cs, h)] = t

    def _cs_dma(cs):
        sc = slice(cs * 512, (cs + 1) * 512)
        nc.sync.dma_start(out=cos_sb[:, sc], in_=cosT[:, sc])
        nc.sync.dma_start(out=sin_sb[:, sc], in_=sinT[:, sc])

    xcs = {}
    for h in range(NHT):
        nc.sync.dma_start(out=wq_sb[:, h * 512:(h + 1) * 512],
                          in_=wq[h * 128:(h + 1) * 128, :])
        _x_dma(0, h)
        nc.sync.dma_start(out=wk_sb[:, h * 128:(h + 1) * 128],
                          in_=wk[h * 128:(h + 1) * 128, :])
        nc.sync.dma_start(out=wv_sb[:, h * 128:(h + 1) * 128],
                          in_=wv[h * 128:(h + 1) * 128, :])
        if h == 12:
            _cs_dma(0)  # rope chunk0 tables needed ~35us in
    for h in range(NHT):
        _x_dma(1, h)
    _cs_dma(1)
    for h in range(NHT):
        _x_dma(2, h)
    _cs_dma(2)
    _cs_dma(3)
    for h in range(NHT):
        _x_dma(3, h)

    for cs in range(NSC):
        sc = slice(cs * 512, (cs + 1) * 512)

        def _rope(dst, pp):
            t1 = rpool.tile([HALF, 512], F32, tag="t1")
            t2 = rpool.tile([HALF, 512], F32, tag="t2")
            nc.vector.tensor_mul(t1[:], pp[0:HALF, :], cos_sb[0:HALF, sc])
            nc.vector.tensor_mul(t2[:], pp[HALF:128, :], sin_sb[0:HALF, sc])
            nc.vector.tensor_sub(dst[0:HALF, sc], t1[:], t2[:])
            t3 = rpool.tile([HALF, 512], F32, tag="t1")
            t4 = rpool.tile([HALF, 512], F32, tag="t2")
            nc.vector.tensor_mul(t3[:], pp[HALF:128, :], cos_sb[HALF:128, sc])
            nc.vector.tensor_mul(t4[:], pp[0:HALF, :], sin_sb[HALF:128, sc])
            nc.vector.tensor_add(dst[HALF:128, sc], t3[:], t4[:])

        for j in range(NREP):
            pq = psum.tile([128, 512], F32, tag="mm")
            for h in range(NHT):
                nc.tensor.matmul(
                    pq[:],
                    wq_sb[:, h * 512 + j * 128: h * 512 + (j + 1) * 128],
                    xcs[(cs, h)][:],
                    start=(h == 0), stop=(h == NHT - 1))
            _rope(qT_sb[j], pq)

        pk = psum.tile([128, 512], F32, tag="mm")
        for h in range(NHT):
            nc.tensor.matmul(pk[:], wk_sb[:, h * 128:(h + 1) * 128],
                             xcs[(cs, h)][:],
                             start=(h == 0), stop=(h == NHT - 1))
        _rope(kT_sb, pk)

        pv = psum.tile([128, 512], F32, tag="mm")
        for tl in range(4):
            for h in range(NHT):
                nc.tensor.matmul(
                    pv[:, tl * 128:(tl + 1) * 128],
                    xcs[(cs, h)][:, tl * 128:(tl + 1) * 128],
                    wv_sb[:, h * 128:(h + 1) * 128],
                    start=(h == 0), stop=(h == NHT - 1))
        nc.scalar.copy(v_sb[:, sc], pv[:])


def _phase2(nc, tc, qT_sb, kT_sb, v_sb, tri_sb, ones_sb,
            ag_in, ag_out, ag3_in, ag3_out, agq, agpool,
            ppool, avcpool, spool, psum, psav, psrs):
    Exp = mybir.ActivationFunctionType.Exp
    carry = [None]

    def issue_agt(j, qq):
        # sync queue, at loop points where AG_j is (nearly) complete, so the
        # in-order queue never blocks anything urgent behind these
        for r in range(N_CORES):
            t = agpool.tile([128, 512], BF16, tag=f"ag{j}_{r}",
                            name=f"ag{j}_{r}_{qq}")
            nc.sync.dma_start(out=t[:],
                              in_=ag_out[j][r * 128:(r + 1) * 128,
                                            qq * 512:(qq + 1) * 512])
            agq[(j, r, qq)] = t

    def make_norm(j, C, pav, prs):
        def f():
            qc = slice(C * 512, (C + 1) * 512)
            # prs already has the rowsum broadcast on all 128 partitions
            bsb = spool.tile([128, 512], F32, tag="bsb", name=f"bsb{j}_{C}")
            nc.vector.reciprocal(bsb[:], prs[:])
            avc = avcpool.tile([128, 512], BF16, tag="avc", name=f"avc{j}_{C}")
            nc.vector.tensor_mul(avc[:], pav[:], bsb[:])
            if j < 3:
                nc.gpsimd.dma_start(out=ag_in[j][:, qc], in_=avc[:])
                if C == NSC - 1:
                    nc.gpsimd.collective_compute(
                        "AllGather", mybir.AluOpType.bypass,
                        replica_groups=[list(range(N_CORES))],
                        ins=[ag_in[j][:]], outs=[ag_out[j][:]])
            else:
                nc.gpsimd.dma_start(out=ag3_in[C][:], in_=avc[:])
                nc.gpsimd.collective_compute(
                    "AllGather", mybir.AluOpType.bypass,
                    replica_groups=[list(range(N_CORES))],
                    ins=[ag3_in[C][:]], outs=[ag3_out[C][:]])
                for r in range(N_CORES):
                    t = agpool.tile([128, 512], BF16, tag=f"ag3_{r}",
                                    name=f"ag3_{r}_{C}")
                    nc.sync.dma_start(
                        out=t[:], in_=ag3_out[C][r * 128:(r + 1) * 128, :])
                    agq[(3, r, C)] = t
        return f

    # agt prefetch schedule: (j, C) loop position -> quarters safe to issue
    prefetch = {(2, 0): [(0, 0)], (2, 2): [(0, 1)],
                (3, 0): [(1, 0)], (3, 1): [(1, 1)],
                (3, 2): [(2, 0)], (3, 3): [(2, 1)]}

    for j in range(NREP):
        for C in range(NSC):
            for (jj, qq) in prefetch.get((j, C), ()):
                issue_agt(jj, qq)
            qc0 = C * 512
            nkt = 4 * C + 4
            prs = psrs.tile([128, 512], F32, tag="rs", name=f"prs{j}_{C}")
            pav = psav.tile([128, 512], F32, tag="av", name=f"pav{j}_{C}")
            pend = []

            def drain_one():
                kt2, off2, pt2 = pend.pop(0)
                nc.tensor.matmul(prs[:, off2:512], ones_sb[:],
                                 pt2[:, off2:512],
                                 start=(kt2 == 0), stop=(kt2 == nkt - 1),
                                 skip_group_check=True)
                nc.tensor.matmul(pav[:, off2:512],
                                 v_sb[:, kt2 * 128:(kt2 + 1) * 128],
                                 pt2[:, off2:512],
                                 start=(kt2 == 0), stop=(kt2 == nkt - 1),
                                 skip_group_check=True)

            for kt in range(nkt):
                off = max(0, (kt - 4 * C) * 128)  # cols < off fully masked
                ps = psum.tile([128, 512], F32, tag="mm",
                               name=f"ps{j}_{C}_{kt}")
                nc.tensor.matmul(ps[:, off:512],
                                 kT_sb[:, kt * 128:(kt + 1) * 128],
                                 qT_sb[j][:, qc0 + off: qc0 + 512],
                                 start=True, stop=True)
                pt = ppool.tile([128, 512], BF16, tag="pt",
                                name=f"pt{j}_{C}_{kt}")
                nc.scalar.activation(pt[:, off:512], ps[:, off:512], Exp)
                if kt >= 4 * C:
                    nc.vector.tensor_mul(pt[:, off:off + 128],
                                         pt[:, off:off + 128], tri_sb[:])
                pend.append((kt, off, pt))
                if kt == 1 and carry[0] is not None:
                    carry[0]()
                    carry[0] = None
                if len(pend) > 3:
                    drain_one()
            while pend:
                drain_one()
            carry[0] = make_norm(j, C, pav, prs)
    carry[0]()
    carry[0] = None


def _phase3(nc, tc, wo_sb, o_out, ag_out, agq, agpool, psum, opool):
    def issue_quarter(qq):
        for j in range(3):
            for r in range(N_CORES):
                t = agpool.tile([128, 512], BF16, tag=f"ag{j}_{r}",
                                name=f"ag{j}_{r}_{qq}")
                nc.sync.dma_start(
                    out=t[:],
                    in_=ag_out[j][r * 128:(r + 1) * 128,
                                  qq * 512:(qq + 1) * 512])
                agq[(j, r, qq)] = t

    issue_quarter(2)
    for st in range(NST):
        if st == 6:
            issue_quarter(3)
        qq = st // 4
        c = st % 4
        po = psum.tile([128, 512], F32, tag="mm", name=f"po{st}")
        i = 0
        for j in range(NREP):
            for r in range(N_CORES):
                t = agq[(j, r, qq)]
                nc.tensor.matmul(po[:], t[:, c * 128:(c + 1) * 128],
                                 wo_sb[:, i * 512:(i + 1) * 512],
                                 start=(i == 0), stop=(i == NHT - 1))
                i += 1
        osb = opool.tile([128, 512], F32, tag="o", name=f"o{st}")
        nc.scalar.copy(osb[:], po[:])
        nc.sync.dma_start(out=o_out[st * 128:(st + 1) * 128, :], in_=osb[:])


def prep_inputs(hidden_states, wq, wk, wv, wo, cos, sin, causal_mask=None):
    bf16 = ml_dtypes.bfloat16
    x = np.asarray(hidden_states, np.float32)[0]          # (S, HID)
    xT = np.ascontiguousarray(x.T).astype(bf16)           # (HID, S)
    wq_s = (np.asarray(wq, np.float32) / np.sqrt(HD)).astype(np.float32)
    cos2 = np.asarray(cos, np.float32)[0, 0]              # (S, 64)
    sin2 = np.asarray(sin, np.float32)[0, 0]
    cosT = np.ascontiguousarray(np.concatenate([cos2.T, cos2.T], 0))  # (128, S)
    sinT = np.ascontiguousarray(np.concatenate([sin2.T, sin2.T], 0))
    kl = np.arange(128)[:, None]
    ql = np.arange(128)[None, :]
    triT = (kl <= ql).astype(bf16)                        # allow k <= q
    ones_c = np.ones((128, 1), bf16)
    ones_r = np.ones((1, 128), np.float32)

    # wo reordered to match AllGather row order: row p = j*1024 + r*128 + d
    # corresponds to head (j*8+r), dim d  ->  wo column (j*8+r)*128 + d.
    j_ = np.arange(NREP)[:, None, None]
    r_ = np.arange(N_CORES)[None, :, None]
    d_ = np.arange(HD)[None, None, :]
    col_order = ((j_ * N_CORES + r_) * HD + d_).reshape(-1)
    woT_full = np.ascontiguousarray(
        np.asarray(wo, np.float32)[:, col_order].T).astype(bf16)  # (4096c, 4096hid)

    in_maps = []
    for c in range(N_CORES):
        heads = [jj * N_CORES + c for jj in range(NREP)]
        wq_rows = np.concatenate([wq_s[h * HD:(h + 1) * HD, :] for h in heads], 0)
        wqT_c = np.ascontiguousarray(wq_rows.T).astype(bf16)        # (HID, 512)
        wkT_c = np.ascontiguousarray(
            np.asarray(wk, np.float32)[c * HD:(c + 1) * HD, :].T).astype(bf16)
        wvT_c = np.ascontiguousarray(
            np.asarray(wv, np.float32)[c * HD:(c + 1) * HD, :].T).astype(bf16)
        woT_c = np.ascontiguousarray(woT_full[:, c * 512:(c + 1) * 512])
        in_maps.append(dict(xT=xT, wqT=wqT_c, wkT=wkT_c, wvT=wvT_c, woT=woT_c,
                            cosT=cosT, sinT=sinT, triT=triT,
                            ones_c=ones_c, ones_r=ones_r))
    return in_maps


def postprocess(results):
    out = np.empty((S, HID), np.float32)
    for c in range(N_CORES):
        out[:, c * 512:(c + 1) * 512] = results[c]["o_out"]
    return out[None]


def get_nc():
    if "nc" not in _CACHE:
        _CACHE["nc"] = build_nc()
    return _CACHE["nc"]


def kernel(hidden_states, wq, wk, wv, wo, cos, sin, causal_mask=None):
    nc = get_nc()
    in_maps = prep_inputs(hidden_states, wq, wk, wv, wo, cos, sin, causal_mask)
    res = run_bass_kernel_spmd(nc, in_maps, core_ids=list(range(N_CORES)))
    return postprocess(res.results)
